# revision 1
# baseline (speedup 1.0000x reference)
"""GroupedQueryAttention TRN2 Bass kernel.

Strategy (8 NeuronCores, tensor-parallel over heads):
  - Each core owns 4 q-heads (one kv head, GQA group of 4).
  - Host pre-transposes x and the weight slices so every matmul operand
    already has its contraction dim on SBUF partitions.
  - Per core: QKV projection (fp32r matmuls), fused RoPE (DVE),
    causal flash-style attention per (batch, head, q-block):
      S^T = K^T.T @ Q^T  -> exp on ACT -> causal mask via gpsimd
      ctx^T = V_aug.T @ exp  (V augmented with a ones column so the
      softmax denominator falls out of the same matmul)
  - AllGather of ctx^T (E/8 rows per core) across the 8 cores, then each
    core computes a 256-column slice of the output projection.
  - Host concatenates + transposes the 8 slices into the full output.
"""

import os
import sys

import numpy as np


def _ensure_concourse():
    try:
        import concourse.bass  # noqa: F401
    except ImportError:
        for p in ("/opt/trn_rl_repo", "/root/.axon_site/_ro/trn_rl_repo"):
            if os.path.isdir(p) and p not in sys.path:
                sys.path.insert(0, p)
        import concourse.bass  # noqa: F401


FULL_CFG = dict(B=2, S=2048, E=2048, NH=32, NKV=8, HD=64, ncores=8, IC=256, IC2=512)

LAST_RESULTS = None  # BassKernelResults of the most recent kernel() call


def build_gqa(cfg):
    """Build the Bass module for one core's SPMD program. Returns nc."""
    _ensure_concourse()
    from contextlib import ExitStack

    import concourse.mybir as mybir
    import concourse.tile as tile
    from concourse import bacc
    from concourse.masks import make_identity

    dt = mybir.dt
    f32 = dt.float32
    f32r = dt.bfloat16 if cfg.get("mmdt", "bf16") == "bf16" else dt.float32r
    Exp = mybir.ActivationFunctionType.Exp

    B, S, E = cfg["B"], cfg["S"], cfg["E"]
    NH, NKV, HD = cfg["NH"], cfg["NKV"], cfg["HD"]
    NCORES = cfg["ncores"]
    HPC = NH // NCORES          # q heads per core
    assert HPC == 4 and HD == 64
    QH = HPC * HD               # 256: per-core q/ctx/out rows
    KVD = 2 * HD                # 128: packed K|V projection width
    NI = B * S                  # total tokens
    ET = E // 128               # contraction tiles
    IC = cfg["IC"]              # phase-1 token chunk
    IC2 = cfg["IC2"]            # phase-5 token chunk
    QB = 512                    # attention q block
    KB = 128                    # attention k block
    NQT = S // QB               # q blocks per batch
    SKT = S // KB               # k blocks per batch
    NKTILES = NI // KB          # total k tiles (both batches)
    scale = 1.0 / float(np.sqrt(HD))

    nc = bacc.Bacc("TRN2", target_bir_lowering=False, debug=False,
                   num_devices=NCORES)

    xT = nc.dram_tensor("xT", [E, NI], f32r, kind="ExternalInput").ap()
    wqT = nc.dram_tensor("wqT", [E, QH], f32r, kind="ExternalInput").ap()
    wkvT = nc.dram_tensor("wkvT", [E, KVD], f32r, kind="ExternalInput").ap()
    woT = nc.dram_tensor("woT", [E, QH], f32r, kind="ExternalInput").ap()
    cosT = nc.dram_tensor("cosT", [128, S], f32, kind="ExternalInput").ap()
    sinT = nc.dram_tensor("sinT", [128, S], f32, kind="ExternalInput").ap()
    outT = nc.dram_tensor("outT", [QH, NI], f32, kind="ExternalOutput").ap()

    with tile.TileContext(nc) as tc, ExitStack() as persist:
        ctxsb_pool = persist.enter_context(tc.tile_pool(name="ctxsb", bufs=1))
        proj_ps = persist.enter_context(
            tc.tile_pool(name="proj_ps", bufs=2, space="PSUM"))
        scores_ps = persist.enter_context(
            tc.tile_pool(name="scores_ps", bufs=2, space="PSUM"))
        ctx_ps_pool = persist.enter_context(
            tc.tile_pool(name="ctx_ps", bufs=2, space="PSUM"))
        dram = persist.enter_context(
            tc.tile_pool(name="dram", bufs=1, space="DRAM"))
        ph13 = persist.enter_context(ExitStack())
        const = ph13.enter_context(tc.tile_pool(name="const", bufs=1))
        qt_pool = ph13.enter_context(tc.tile_pool(name="qt", bufs=1))
        kt_pool = ph13.enter_context(tc.tile_pool(name="kt", bufs=1))
        vaug_pool = ph13.enter_context(tc.tile_pool(name="vaug", bufs=1))

        ident = const.tile([128, 128], f32, name="ident", tag="ident")
        make_identity(nc, ident[:, :])
        cos_sb = const.tile([128, S], f32, name="cos_sb", tag="cos")
        nc.sync.dma_start(cos_sb[:, :], cosT)
        sin_sb = const.tile([128, S], f32, name="sin_sb", tag="sin")
        nc.sync.dma_start(sin_sb[:, :], sinT)
        wq_sb = const.tile([128, ET, QH], f32r, name="wq_sb", tag="wq")
        nc.sync.dma_start(wq_sb[:, :, :],
                          wqT.rearrange("(t p) o -> p t o", p=128))
        wkv_sb = const.tile([128, ET, KVD], f32r, name="wkv_sb", tag="wkv")
        nc.sync.dma_start(wkv_sb[:, :, :],
                          wkvT.rearrange("(t p) o -> p t o", p=128))
        ones_col = const.tile([128, 1], f32, name="ones_col", tag="ones")
        nc.gpsimd.memset(ones_col[:, :], 1.0)
        nqb = QB // KB
        mask_sb = const.tile([128, nqb, QB], f32r, name="mask_sb", tag="mask")
        nc.gpsimd.memset(mask_sb[:, :, :], 1.0)
        for j in range(nqb):
            # keep where dq >= dk + KB*j, else 0 (causal within diagonal band)
            nc.gpsimd.affine_select(
                out=mask_sb[:, j, :], in_=mask_sb[:, j, :],
                pattern=[[1, QB]], compare_op=mybir.AluOpType.is_ge,
                fill=0.0, base=-KB * j, channel_multiplier=-1)

        # persistent activations
        qt_sb = [qt_pool.tile([128, NI], f32r, name=f"qt{m}", tag=f"qt{m}")
                 for m in range(HPC // 2)]
        kt_sb = kt_pool.tile([128, NI], f32r, tag="ktd")  # K^T duplicated 2x
        vaug = [vaug_pool.tile([128, HD + 1], f32r, name=f"va{k}", tag=f"va{k}")
                for k in range(NKTILES)]
        ctx_sb = [ctxsb_pool.tile([128, NI], f32r, name=f"cx{m}", tag=f"cx{m}")
                  for m in range(HPC // 2)]

        def rope(dst, src_ps, parts, s0, ln, qs_t, t1_t):
            # dst = src*cos + swap(src)*signed_sin ; src is PSUM, dst SBUF
            for h0 in range(0, parts, 64):
                nc.vector.tensor_copy(qs_t[h0:h0 + 32, :ln],
                                      src_ps[h0 + 32:h0 + 64, :ln])
                nc.vector.tensor_copy(qs_t[h0 + 32:h0 + 64, :ln],
                                      src_ps[h0:h0 + 32, :ln])
            nc.vector.tensor_mul(t1_t[:parts, :ln], src_ps[:parts, :ln],
                                 cos_sb[:parts, s0:s0 + ln])
            nc.vector.tensor_mul(qs_t[:parts, :ln], qs_t[:parts, :ln],
                                 sin_sb[:parts, s0:s0 + ln])
            nc.vector.tensor_add(dst, t1_t[:parts, :ln], qs_t[:parts, :ln])

        # ---- phase 1-3 scoped pools
        if True:
            xt_pool = ph13.enter_context(tc.tile_pool(name="xt", bufs=2))
            rope_pool = ph13.enter_context(tc.tile_pool(name="rope", bufs=2))
            vs_pool = ph13.enter_context(tc.tile_pool(name="vs", bufs=2))
            exp_pool = ph13.enter_context(tc.tile_pool(name="exp", bufs=3))
            rc_pool = ph13.enter_context(tc.tile_pool(name="rc", bufs=2))
            rb_pool = ph13.enter_context(tc.tile_pool(name="rb", bufs=2))

            # ---- phase 1: QKV projection + RoPE + V transpose
            for ch in range(NI // IC):
                i0 = ch * IC
                s0 = i0 % S
                xt = xt_pool.tile([128, ET, IC], f32r, name="xt", tag="xt")
                nc.sync.dma_start(
                    xt[:, :, :],
                    xT[:, i0:i0 + IC].rearrange("(t p) i -> p t i", p=128))
                for m in range(HPC // 2):
                    q_ps = proj_ps.tile([128, IC], f32, name="pps", tag="proj")
                    for t in range(ET):
                        nc.tensor.matmul(
                            q_ps[:, :],
                            wq_sb[:, t, m * 128:(m + 1) * 128],
                            xt[:, t, :],
                            start=(t == 0), stop=(t == ET - 1))
                    qs_t = rope_pool.tile([128, IC], f32, name="qs_t", tag="qs")
                    t1_t = rope_pool.tile([128, IC], f32, name="t1_t", tag="t1")
                    rope(qt_sb[m][:, i0:i0 + IC], q_ps, 128, s0, IC, qs_t, t1_t)
                kv_ps = proj_ps.tile([128, IC], f32, name="pps", tag="proj")
                for t in range(ET):
                    nc.tensor.matmul(
                        kv_ps[:, :],
                        wkv_sb[:, t, :],
                        xt[:, t, :],
                        start=(t == 0), stop=(t == ET - 1))
                qs_t = rope_pool.tile([128, IC], f32, name="qs_t", tag="qs")
                t1_t = rope_pool.tile([128, IC], f32, name="t1_t", tag="t1")
                rope(kt_sb[0:64, i0:i0 + IC], kv_ps, 64, s0, IC, qs_t, t1_t)
                nc.vector.tensor_copy(kt_sb[64:128, i0:i0 + IC],
                                      kt_sb[0:64, i0:i0 + IC])
                vs = vs_pool.tile([64, IC], f32, name="vs", tag="vs")
                nc.vector.tensor_copy(vs[:, :], kv_ps[64:128, :])
                for j in range(IC // 128):
                    kidx = (i0 + j * 128) // 128
                    vt_ps = scores_ps.tile([128, 2 * QB], f32, name="s_ps", tag="s")
                    nc.tensor.transpose(vt_ps[:, 0:64], vs[:, j * 128:(j + 1) * 128],
                                        ident[0:64, 0:64])
                    nc.vector.tensor_copy(vaug[kidx][:, 0:HD], vt_ps[:, 0:64])
                    nc.vector.tensor_copy(vaug[kidx][:, HD:HD + 1], ones_col[:, :])

            # ---- phase 3: attention
            for b in range(B):
                for qt in range(NQT):
                    for h in range(HPC):
                        mt, hb = h // 2, (h % 2) * 64
                        q_ap = qt_sb[mt][hb:hb + 64, b * S + qt * QB:
                                         b * S + qt * QB + QB]
                        ctx_ps = ctx_ps_pool.tile([128, QB], f32, name="ctx_ps", tag="ctx")
                        nkt = (qt + 1) * (QB // KB)
                        for kp in range(nkt // 2):
                            s_ps = scores_ps.tile([128, 2 * QB], f32, name="s_ps", tag="s")
                            e_t = exp_pool.tile([128, 2 * QB], f32r, name="e_t", tag="e")
                            for hf in range(2):
                                kt = kp * 2 + hf
                                nc.tensor.matmul(
                                    s_ps[:, hf * QB:(hf + 1) * QB],
                                    kt_sb[hb:hb + 64,
                                          b * S + kt * KB:b * S + kt * KB + KB
                                          ],
                                    q_ap,
                                    start=True, stop=True)
                            nc.scalar.activation(e_t[:, :], s_ps[:, :], Exp,
                                                 scale=scale)
                            for hf in range(2):
                                kt = kp * 2 + hf
                                j = kt - qt * (QB // KB)
                                if j >= 0:  # diagonal block: causal mask
                                    nc.vector.tensor_mul(
                                        e_t[:, hf * QB:(hf + 1) * QB],
                                        e_t[:, hf * QB:(hf + 1) * QB],
                                        mask_sb[:, j, :])
                                nc.tensor.matmul(
                                    ctx_ps[0:HD + 1, :],
                                    vaug[b * SKT + kt][:, :],
                                    e_t[:, hf * QB:(hf + 1) * QB],
                                    start=(kt == 0), stop=(kt == nkt - 1))
                        recip = rc_pool.tile([1, QB], f32, name="recip", tag="rc")
                        nc.vector.reciprocal(recip[0:1, :], ctx_ps[HD:HD + 1, :])
                        rbc = rb_pool.tile([64, QB], f32, name="rbc", tag="rb")
                        nc.gpsimd.partition_broadcast(rbc[:, :], recip[0:1, :])
                        nc.vector.tensor_mul(
                            ctx_sb[mt][hb:hb + 64,
                                       b * S + qt * QB:b * S + qt * QB + QB],
                            ctx_ps[0:HD, :], rbc[:, :])

        ph13.close()

        # ---- phase 4: AllGather ctx^T across cores
        cc_in = dram.tile([QH, NI], f32r, name="cc_in", tag="ccin")
        for m in range(HPC // 2):
            nc.sync.dma_start(cc_in[m * 128:(m + 1) * 128, :], ctx_sb[m][:, :])
        cc_out = dram.tile([E, NI], f32r, name="cc_out", tag="ccout",
                           addr_space="Shared" if NCORES > 4 else "Local")
        nc.gpsimd.collective_compute(
            "AllGather", mybir.AluOpType.bypass,
            replica_groups=[list(range(NCORES))],
            ins=[cc_in[:, :]],
            outs=[cc_out[:, :]])

        # ---- phase 5: output projection slice
        with ExitStack() as ph5:
            wo_pool = ph5.enter_context(tc.tile_pool(name="wo", bufs=1))
            ct_pool = ph5.enter_context(tc.tile_pool(name="ct", bufs=2))
            ob_pool = ph5.enter_context(tc.tile_pool(name="ob", bufs=2))
            wo_sb = wo_pool.tile([128, ET, QH], f32r, name="wo_sb", tag="wo")
            nc.sync.dma_start(wo_sb[:, :, :],
                              woT.rearrange("(t p) o -> p t o", p=128))
            for ch in range(NI // IC2):
                i0 = ch * IC2
                ct = ct_pool.tile([128, ET, IC2], f32r, name="ct", tag="ct")
                nc.sync.dma_start(
                    ct[:, :, :],
                    cc_out[:, i0:i0 + IC2].rearrange("(t p) i -> p t i", p=128))
                for m in range(HPC // 2):
                    o_ps = proj_ps.tile([128, IC2], f32, name="ops", tag="proj")
                    for t in range(ET):
                        nc.tensor.matmul(
                            o_ps[:, :],
                            wo_sb[:, t, m * 128:(m + 1) * 128],
                            ct[:, t, :],
                            start=(t == 0), stop=(t == ET - 1))
                    ob = ob_pool.tile([128, IC2], f32, name="ob", tag="ob")
                    nc.vector.tensor_copy(ob[:, :], o_ps[:, :])
                    nc.sync.dma_start(outT[m * 128:(m + 1) * 128, i0:i0 + IC2],
                                      ob[:, :])

    nc.compile()
    return nc


def make_in_maps(cfg, x, cos, sin, Wq, Wk, Wv, Wo):
    """Host-side prep: transpose/slice full inputs into per-core input maps."""
    B, S, E = cfg["B"], cfg["S"], cfg["E"]
    NH, NKV, HD, NCORES = cfg["NH"], cfg["NKV"], cfg["HD"], cfg["ncores"]
    HPC = NH // NCORES
    QH = HPC * HD
    KVPC = NKV // NCORES

    x = np.asarray(x, dtype=np.float32)
    cos = np.asarray(cos, dtype=np.float32)
    sin = np.asarray(sin, dtype=np.float32)
    Wq = np.asarray(Wq, dtype=np.float32)
    Wk = np.asarray(Wk, dtype=np.float32)
    Wv = np.asarray(Wv, dtype=np.float32)
    Wo = np.asarray(Wo, dtype=np.float32)

    if cfg.get("mmdt", "bf16") == "bf16":
        import ml_dtypes
        mmnp = ml_dtypes.bfloat16
    else:
        mmnp = np.float32
    xT = np.ascontiguousarray(x.reshape(B * S, E).T.astype(mmnp))
    cos_t = cos.T[:HD]                        # [64, S]
    cosT = np.ascontiguousarray(np.concatenate([cos_t, cos_t], axis=0))
    sin_t = sin.T[:HD].copy()
    sin_t[:HD // 2] *= -1.0                   # signed sin for rotate-half
    sinT = np.ascontiguousarray(np.concatenate([sin_t, sin_t], axis=0))

    in_maps = []
    for c in range(NCORES):
        qsl = slice(c * QH, (c + 1) * QH)
        ksl = slice(c * KVPC * HD, (c + 1) * KVPC * HD)
        wq = np.ascontiguousarray(Wq[qsl, :].T.astype(mmnp))
        wkv = np.ascontiguousarray(
            np.concatenate([Wk[ksl, :].T, Wv[ksl, :].T], axis=1).astype(mmnp))
        wo = np.ascontiguousarray(Wo[qsl, :].T.astype(mmnp))
        in_maps.append(dict(xT=xT, wqT=wq, wkvT=wkv, woT=wo,
                            cosT=cosT, sinT=sinT))
    return in_maps


def assemble_output(cfg, results):
    B, S, E = cfg["B"], cfg["S"], cfg["E"]
    outT = np.concatenate([r["outT"] for r in results], axis=0)  # [E, B*S]
    return np.ascontiguousarray(outT.T.reshape(B, S, E).astype(np.float32))


def kernel(x, mask, cos, sin, Wq, Wk, Wv, Wo):
    global LAST_RESULTS
    _ensure_concourse()
    from concourse import bass_utils

    cfg = FULL_CFG
    nc = build_gqa(cfg)
    in_maps = make_in_maps(cfg, x, cos, sin, Wq, Wk, Wv, Wo)
    res = bass_utils.run_bass_kernel_spmd(
        nc, in_maps, core_ids=list(range(cfg["ncores"])))
    LAST_RESULTS = res
    return assemble_output(cfg, res.results)



# revision 6
# speedup vs baseline: 1.1090x; 1.1090x over previous
"""GroupedQueryAttention TRN2 Bass kernel (v2).

Strategy (8 NeuronCores, tensor-parallel over heads):
  - Each core owns 4 q-heads (one kv head, GQA group of 4), all tokens.
  - Phase 1: QKV projection (bf16 matmuls, N=512 chunks) + fused RoPE.
    Q stored [64, 4 heads, NI] so scores batch 2 heads per matmul.
  - Phase 3: causal flash-style attention per (head-pair, batch, q-stripe):
      S = K_blk^T.T @ Q(2 heads)  -> exp on ACT (causally trimmed)
      ctx^T += V_aug.T @ exp  (V augmented with ones column so the softmax
      denominator falls out of the same matmul); normalize via reciprocal +
      partition broadcast fused into the bf16 staging store.
  - Phase 4: two 1 MB AllToAlls (one per head-pair) re-shard from
    head-sharded to token-sharded; the first overlaps pair-1 attention.
  - Phase 5: out = ctx_tok^T stationary x full-Wo moving (N=2048 matmuls),
    each core emits out[token-slice 512, 2048].
  - Host concatenates the 8 token slices.
"""

import os
import sys

import numpy as np


def _ensure_concourse():
    try:
        import concourse.bass  # noqa: F401
    except ImportError:
        for p in ("/opt/trn_rl_repo", "/root/.axon_site/_ro/trn_rl_repo"):
            if os.path.isdir(p) and p not in sys.path:
                sys.path.insert(0, p)
        import concourse.bass  # noqa: F401


FULL_CFG = dict(B=2, S=2048, E=2048, NH=32, NKV=8, HD=64, ncores=8, IC=512)

LAST_RESULTS = None
_CACHED_NC = None


def build_gqa(cfg):
    """Build the Bass module for one core's SPMD program. Returns nc."""
    _ensure_concourse()
    from contextlib import ExitStack

    import concourse.mybir as mybir
    import concourse.tile as tile
    from concourse import bacc
    from concourse.masks import make_identity

    dt = mybir.dt
    f32 = dt.float32
    bf16 = dt.bfloat16
    Exp = mybir.ActivationFunctionType.Exp

    B, S, E = cfg["B"], cfg["S"], cfg["E"]
    NH, NKV, HD = cfg["NH"], cfg["NKV"], cfg["HD"]
    NCORES = cfg["ncores"]
    HPC = NH // NCORES          # 4 q heads per core
    assert HPC == 4 and HD == 64
    QH = HPC * HD               # 256 ctx rows per core
    KVD = 2 * HD                # 128 packed K|V projection width
    NI = B * S                  # 4096 tokens
    ET = E // 128               # 16 contraction tiles
    IC = cfg["IC"]              # phase-1 token chunk (512)
    QB = 512                    # attention q stripe
    KB = 128                    # attention k block
    NQT = S // QB               # 4 stripes per batch
    SKT = S // KB               # 16 k tiles per batch
    NKTILES = NI // KB          # 32 k tiles
    TOK = NI // NCORES          # 512-token output slice per core
    scale = 1.0 / float(np.sqrt(HD))

    nc = bacc.Bacc("TRN2", target_bir_lowering=False, debug=False,
                   num_devices=NCORES)

    xT = nc.dram_tensor("xT", [E, NI], bf16, kind="ExternalInput").ap()
    wqT = nc.dram_tensor("wqT", [E, QH], bf16, kind="ExternalInput").ap()
    wkvT = nc.dram_tensor("wkvT", [E, KVD], bf16, kind="ExternalInput").ap()
    woT = nc.dram_tensor("woT", [E, E], bf16, kind="ExternalInput").ap()
    cosT = nc.dram_tensor("cosT", [128, S], bf16, kind="ExternalInput").ap()
    sinT = nc.dram_tensor("sinT", [128, S], bf16, kind="ExternalInput").ap()
    outT = nc.dram_tensor("outT", [TOK, E], bf16, kind="ExternalOutput").ap()

    with tile.TileContext(nc) as tc, ExitStack() as persist:
        const = persist.enter_context(tc.tile_pool(name="const", bufs=1))
        qt_pool = persist.enter_context(tc.tile_pool(name="qt", bufs=1))
        kt_pool = persist.enter_context(tc.tile_pool(name="kt", bufs=1))
        vaug_pool = persist.enter_context(tc.tile_pool(name="vaug", bufs=1))
        dram = persist.enter_context(
            tc.tile_pool(name="dram", bufs=1, space="DRAM"))

        ident = const.tile([128, 128], bf16, name="ident", tag="ident")
        make_identity(nc, ident[:, :])
        cos_sb = const.tile([128, S], bf16, name="cos_sb", tag="cos")
        nc.sync.dma_start(cos_sb[:, :], cosT)
        sin_sb = const.tile([128, S], bf16, name="sin_sb", tag="sin")
        nc.sync.dma_start(sin_sb[:, :], sinT)
        wq_sb = const.tile([128, ET, QH], bf16, name="wq_sb", tag="wq")
        nc.sync.dma_start(wq_sb[:, :, :],
                          wqT.rearrange("(t p) o -> p t o", p=128))
        wkv_sb = const.tile([128, ET, KVD], bf16, name="wkv_sb", tag="wkv")
        nc.sync.dma_start(wkv_sb[:, :, :],
                          wkvT.rearrange("(t p) o -> p t o", p=128))
        wo_sb = const.tile([128, ET, E], bf16, name="wo_sb", tag="wo")
        nc.sync.dma_start(wo_sb[:, :, :],
                          woT.rearrange("(t p) o -> p t o", p=128))
        # triangular causal mask for the diagonal 128-block, dup for 2 heads
        tri = const.tile([128, 2, 128], bf16, name="tri", tag="tri")
        nc.gpsimd.memset(tri[:, :, :], 1.0)
        nc.gpsimd.affine_select(
            out=tri[:, :, :], in_=tri[:, :, :],
            pattern=[[0, 2], [1, 128]], compare_op=mybir.AluOpType.is_ge,
            fill=0.0, base=0, channel_multiplier=-1)

        # persistent activations
        qt_sb = qt_pool.tile([64, HPC, NI], bf16, name="qt", tag="qt")
        kt_sb = kt_pool.tile([64, NI], bf16, name="kt", tag="kt")
        vaug = [vaug_pool.tile([128, HD + 1], bf16, name=f"va{k}",
                               tag=f"va{k}")
                for k in range(NKTILES)]
        for k in range(NKTILES):
            nc.vector.memset(vaug[k][:, HD:HD + 1], 1.0)

        # collective buffers: per head-pair m, [slice, 128 rows, 512 tokens]
        cc_in = [dram.tile([NCORES, 128, TOK], bf16, name=f"cc_in{m}",
                           tag=f"ccin{m}") for m in range(2)]
        cc_out = [dram.tile([NCORES, 128, TOK], bf16, name=f"cc_out{m}",
                            tag=f"ccout{m}") for m in range(2)]

        # ---- phase 1: QKV projection + RoPE + V transpose
        with ExitStack() as ph1:
            xt_pool = ph1.enter_context(tc.tile_pool(name="xt", bufs=3))
            proj_ps = ph1.enter_context(
                tc.tile_pool(name="proj_ps", bufs=3, space="PSUM"))
            vt_ps_pool = ph1.enter_context(
                tc.tile_pool(name="vt_ps", bufs=2, space="PSUM"))
            rope_pool = ph1.enter_context(tc.tile_pool(name="rope", bufs=3))
            vs_pool = ph1.enter_context(tc.tile_pool(name="vs", bufs=2))

            def rope(src_ps, parts, s0, dsts):
                # dsts: list of (out_ap, row0) pairs covering src rows
                t1 = rope_pool.tile([128, IC], bf16, name="t1", tag="t1")
                sw = rope_pool.tile([128, IC], bf16, name="sw", tag="sw")
                for h0 in range(0, parts, 64):
                    nc.scalar.copy(sw[h0:h0 + 32, :],
                                   src_ps[h0 + 32:h0 + 64, :])
                    nc.scalar.copy(sw[h0 + 32:h0 + 64, :],
                                   src_ps[h0:h0 + 32, :])
                nc.vector.tensor_mul(t1[:parts, :], src_ps[:parts, :],
                                     cos_sb[:parts, s0:s0 + IC])
                nc.vector.tensor_mul(sw[:parts, :], sw[:parts, :],
                                     sin_sb[:parts, s0:s0 + IC])
                for out_ap, r0 in dsts:
                    nc.vector.tensor_add(out_ap, t1[r0:r0 + 64, :],
                                         sw[r0:r0 + 64, :])

            for ch in range(NI // IC):
                i0 = ch * IC
                s0 = i0 % S
                xt = xt_pool.tile([128, ET, IC], bf16, name="xt", tag="xt")
                nc.sync.dma_start(
                    xt[:, :, :],
                    xT[:, i0:i0 + IC].rearrange("(t p) i -> p t i", p=128))
                for m in range(2):
                    q_ps = proj_ps.tile([128, IC], f32, name="pps",
                                        tag="proj")
                    for t in range(ET):
                        nc.tensor.matmul(
                            q_ps[:, :],
                            wq_sb[:, t, m * 128:(m + 1) * 128],
                            xt[:, t, :],
                            start=(t == 0), stop=(t == ET - 1))
                    rope(q_ps, 128, s0,
                         [(qt_sb[0:64, 2 * m, i0:i0 + IC], 0),
                          (qt_sb[0:64, 2 * m + 1, i0:i0 + IC], 64)])
                kv_ps = proj_ps.tile([128, IC], f32, name="pps", tag="proj")
                for t in range(ET):
                    nc.tensor.matmul(
                        kv_ps[:, :],
                        wkv_sb[:, t, :],
                        xt[:, t, :],
                        start=(t == 0), stop=(t == ET - 1))
                rope(kv_ps, 64, s0, [(kt_sb[0:64, i0:i0 + IC], 0)])
                vs = vs_pool.tile([64, IC], bf16, name="vs", tag="vs")
                nc.scalar.copy(vs[:, :], kv_ps[64:128, :])
                for j in range(IC // 128):
                    kidx = (i0 + j * 128) // 128
                    vt_ps = vt_ps_pool.tile([128, HD], bf16, name="vt",
                                            tag="vt")
                    nc.tensor.transpose(vt_ps[:, :],
                                        vs[:, j * 128:(j + 1) * 128],
                                        ident[0:64, 0:64])
                    nc.vector.tensor_copy(vaug[kidx][:, 0:HD], vt_ps[:, :])

        # ---- phase 3: attention (head-pair outer so the pair-0 AllToAll
        #      overlaps pair-1 compute)
        with ExitStack() as ph3:
            scores_ps = ph3.enter_context(
                tc.tile_pool(name="scores_ps", bufs=2, space="PSUM"))
            ctx_ps_pool = ph3.enter_context(
                tc.tile_pool(name="ctx_ps", bufs=2, space="PSUM"))
            et_pool = ph3.enter_context(tc.tile_pool(name="et", bufs=3))
            rc_pool = ph3.enter_context(tc.tile_pool(name="rc", bufs=2))
            rb_pool = ph3.enter_context(tc.tile_pool(name="rb", bufs=2))
            st_pool = ph3.enter_context(tc.tile_pool(name="st", bufs=2))

            for m in range(2):
                for b in range(B):
                    for qt in range(NQT):
                        sl = b * S + qt * QB
                        nkt = (qt + 1) * (QB // KB)
                        ctx_ps = ctx_ps_pool.tile([HD + 1, 2, QB], f32,
                                                  name="ctx", tag="ctx")
                        for kt in range(nkt):
                            j = kt - qt * (QB // KB)
                            kp = b * S + kt * KB
                            s_ps = scores_ps.tile([128, 2, QB], f32,
                                                  name="sps", tag="sps")
                            e_t = et_pool.tile([128, 2, QB], bf16,
                                               name="et", tag="et")
                            if j < 0:
                                for h in range(2):
                                    nc.tensor.matmul(
                                        s_ps[:, h, :],
                                        kt_sb[0:64, kp:kp + KB],
                                        qt_sb[0:64, 2 * m + h, sl:sl + QB],
                                        start=True, stop=True)
                                nc.scalar.activation(e_t[:, :, :],
                                                     s_ps[:, :, :], Exp,
                                                     scale=scale)
                            else:
                                q0 = j * KB
                                for h in range(2):
                                    nc.tensor.matmul(
                                        s_ps[:, h, q0:QB],
                                        kt_sb[0:64, kp:kp + KB],
                                        qt_sb[0:64, 2 * m + h,
                                              sl + q0:sl + QB],
                                        start=True, stop=True)
                                if j > 0:
                                    nc.gpsimd.memset(e_t[:, :, 0:q0], 0.0)
                                nc.scalar.activation(e_t[:, :, q0:QB],
                                                     s_ps[:, :, q0:QB], Exp,
                                                     scale=scale)
                                nc.vector.tensor_mul(
                                    e_t[:, :, q0:q0 + KB],
                                    e_t[:, :, q0:q0 + KB],
                                    tri[:, :, :])
                            for h in range(2):
                                nc.tensor.matmul(
                                    ctx_ps[:, h, :],
                                    vaug[b * SKT + kt][:, :],
                                    e_t[:, h, :],
                                    start=(kt == 0), stop=(kt == nkt - 1))
                        # normalize by the ones-column row + stage for A2A
                        rc = rc_pool.tile([1, 2, QB], f32, name="rc",
                                          tag="rc")
                        nc.vector.reciprocal(rc[:, :, :],
                                             ctx_ps[HD:HD + 1, :, :])
                        rb = rb_pool.tile([64, 2, QB], f32, name="rb",
                                          tag="rb")
                        nc.gpsimd.partition_broadcast(rb[:, :, :],
                                                      rc[:, :, :])
                        stage = st_pool.tile([128, QB], bf16, name="st",
                                             tag="st")
                        nc.vector.tensor_mul(stage[0:64, :],
                                             ctx_ps[0:HD, 0, :],
                                             rb[:, 0, :])
                        nc.vector.tensor_mul(stage[64:128, :],
                                             ctx_ps[0:HD, 1, :],
                                             rb[:, 1, :])
                        nc.sync.dma_start(cc_in[m][b * NQT + qt, :, :],
                                          stage[:, :])
                # ---- phase 4: AllToAll for this head-pair
                nc.gpsimd.collective_compute(
                    "AllToAll", mybir.AluOpType.bypass,
                    replica_groups=[list(range(NCORES))],
                    ins=[cc_in[m][:, :, :]],
                    outs=[cc_out[m][:, :, :]])

        # ---- phase 5: output projection for this core's 512-token slice
        with ExitStack() as ph5:
            ct_pool = ph5.enter_context(tc.tile_pool(name="ct", bufs=1))
            out_ps_pool = ph5.enter_context(
                tc.tile_pool(name="out_ps", bufs=2, space="PSUM"))
            ob_pool = ph5.enter_context(tc.tile_pool(name="ob", bufs=2))

            ct = [ct_pool.tile([128, NCORES, TOK], bf16, name=f"ct{m}",
                               tag=f"ct{m}") for m in range(2)]
            for m in range(2):
                nc.sync.dma_start(ct[m][:, :, :],
                                  cc_out[m].rearrange("s p n -> p s n"))

            out_ps = [out_ps_pool.tile([128, E], f32, name=f"op{c}",
                                       tag="ops") for c in range(4)]
            order = [(0, 0), (1, 0), (0, 1), (1, 1), (2, 0), (2, 1),
                     (3, 0), (3, 1)]
            done = set()
            for c, m in order:
                for d in range(NCORES):
                    for o in range(E // 512):
                        nc.tensor.matmul(
                            out_ps[c][:, o * 512:(o + 1) * 512],
                            ct[m][:, d, c * 128:(c + 1) * 128],
                            wo_sb[:, 2 * d + m, o * 512:(o + 1) * 512],
                            start=(m == 0 and d == 0),
                            stop=(m == 1 and d == NCORES - 1))
                if m == 1:
                    ob = ob_pool.tile([128, E], bf16, name="ob", tag="ob")
                    nc.vector.tensor_copy(ob[:, :], out_ps[c][:, :])
                    nc.sync.dma_start(outT[c * 128:(c + 1) * 128, :],
                                      ob[:, :])
                    done.add(c)
            assert done == {0, 1, 2, 3}

    nc.compile()
    return nc


def make_in_maps(cfg, x, cos, sin, Wq, Wk, Wv, Wo):
    """Host-side prep: transpose/slice full inputs into per-core maps."""
    import ml_dtypes
    B, S, E = cfg["B"], cfg["S"], cfg["E"]
    NH, NKV, HD, NCORES = cfg["NH"], cfg["NKV"], cfg["HD"], cfg["ncores"]
    HPC = NH // NCORES
    QH = HPC * HD
    KVPC = NKV // NCORES
    bf = ml_dtypes.bfloat16

    x = np.asarray(x, dtype=np.float32)
    cos = np.asarray(cos, dtype=np.float32)
    sin = np.asarray(sin, dtype=np.float32)
    Wq = np.asarray(Wq, dtype=np.float32)
    Wk = np.asarray(Wk, dtype=np.float32)
    Wv = np.asarray(Wv, dtype=np.float32)
    Wo = np.asarray(Wo, dtype=np.float32)

    xT = np.ascontiguousarray(x.reshape(B * S, E).T.astype(bf))
    cos_t = cos.T[:HD]                        # [64, S]
    cosT = np.ascontiguousarray(
        np.concatenate([cos_t, cos_t], axis=0).astype(bf))
    sin_t = sin.T[:HD].copy()
    sin_t[:HD // 2] *= -1.0                   # signed sin for rotate-half
    sinT = np.ascontiguousarray(
        np.concatenate([sin_t, sin_t], axis=0).astype(bf))
    woT = np.ascontiguousarray(Wo.T.astype(bf))  # full [E_in, E_out]

    in_maps = []
    for c in range(NCORES):
        qsl = slice(c * QH, (c + 1) * QH)
        ksl = slice(c * KVPC * HD, (c + 1) * KVPC * HD)
        wq = np.ascontiguousarray(Wq[qsl, :].T.astype(bf))
        wkv = np.ascontiguousarray(
            np.concatenate([Wk[ksl, :].T, Wv[ksl, :].T], axis=1).astype(bf))
        in_maps.append(dict(xT=xT, wqT=wq, wkvT=wkv, woT=woT,
                            cosT=cosT, sinT=sinT))
    return in_maps


def assemble_output(cfg, results):
    B, S, E = cfg["B"], cfg["S"], cfg["E"]
    out = np.concatenate([np.asarray(r["outT"]) for r in results], axis=0)
    return np.ascontiguousarray(out.astype(np.float32).reshape(B, S, E))


def kernel(x, mask, cos, sin, Wq, Wk, Wv, Wo):
    global LAST_RESULTS, _CACHED_NC
    _ensure_concourse()
    from concourse import bass_utils

    cfg = FULL_CFG
    if _CACHED_NC is None:
        _CACHED_NC = build_gqa(cfg)
    nc = _CACHED_NC
    in_maps = make_in_maps(cfg, x, cos, sin, Wq, Wk, Wv, Wo)
    res = bass_utils.run_bass_kernel_spmd(
        nc, in_maps, core_ids=list(range(cfg["ncores"])))
    LAST_RESULTS = res
    return assemble_output(cfg, res.results)


# revision 12
# speedup vs baseline: 1.2546x; 1.1313x over previous
"""GroupedQueryAttention TRN2 Bass kernel (v2).

Strategy (8 NeuronCores, tensor-parallel over heads):
  - Each core owns 4 q-heads (one kv head, GQA group of 4), all tokens.
  - Phase 1: QKV projection (bf16 matmuls, N=512 chunks) + fused RoPE.
    Q stored [64, 4 heads, NI] so scores batch 2 heads per matmul.
  - Phase 3: causal flash-style attention per (head-pair, batch, q-stripe):
      S = K_blk^T.T @ Q(2 heads)  -> exp on ACT (causally trimmed)
      ctx^T += V_aug.T @ exp  (V augmented with ones column so the softmax
      denominator falls out of the same matmul); normalize via reciprocal +
      partition broadcast fused into the bf16 staging store.
  - Phase 4: two 1 MB AllToAlls (one per head-pair) re-shard from
    head-sharded to token-sharded; the first overlaps pair-1 attention.
  - Phase 5: out = ctx_tok^T stationary x full-Wo moving (N=2048 matmuls),
    each core emits out[token-slice 512, 2048].
  - Host concatenates the 8 token slices.
"""

import os
import sys

import numpy as np


def _ensure_concourse():
    try:
        import concourse.bass  # noqa: F401
    except ImportError:
        for p in ("/opt/trn_rl_repo", "/root/.axon_site/_ro/trn_rl_repo"):
            if os.path.isdir(p) and p not in sys.path:
                sys.path.insert(0, p)
        import concourse.bass  # noqa: F401


FULL_CFG = dict(B=2, S=2048, E=2048, NH=32, NKV=8, HD=64, ncores=8, IC=512)

LAST_RESULTS = None
_CACHED_NC = None


def build_gqa(cfg):
    """Build the Bass module for one core's SPMD program. Returns nc."""
    _ensure_concourse()
    from contextlib import ExitStack

    import concourse.mybir as mybir
    import concourse.tile as tile
    from concourse import bacc
    from concourse.masks import make_identity

    dt = mybir.dt
    f32 = dt.float32
    bf16 = dt.bfloat16
    Exp = mybir.ActivationFunctionType.Exp

    B, S, E = cfg["B"], cfg["S"], cfg["E"]
    NH, NKV, HD = cfg["NH"], cfg["NKV"], cfg["HD"]
    NCORES = cfg["ncores"]
    HPC = NH // NCORES          # 4 q heads per core
    assert HPC == 4 and HD == 64
    QH = HPC * HD               # 256 ctx rows per core
    KVD = 2 * HD                # 128 packed K|V projection width
    NI = B * S                  # 4096 tokens
    ET = E // 128               # 16 contraction tiles
    IC = cfg["IC"]              # phase-1 token chunk (512)
    QB = 512                    # attention q stripe
    KB = 128                    # attention k block
    NQT = S // QB               # 4 stripes per batch
    SKT = S // KB               # 16 k tiles per batch
    NKTILES = NI // KB          # 32 k tiles
    TOK = NI // NCORES          # 512-token output slice per core
    scale = 1.0 / float(np.sqrt(HD))

    nc = bacc.Bacc("TRN2", target_bir_lowering=False, debug=False,
                   num_devices=NCORES)

    xT = nc.dram_tensor("xT", [E, NI], bf16, kind="ExternalInput").ap()
    wqT = nc.dram_tensor("wqT", [E, QH], bf16, kind="ExternalInput").ap()
    wkvT = nc.dram_tensor("wkvT", [E, KVD], bf16, kind="ExternalInput").ap()
    woT = nc.dram_tensor("woT", [E, E], bf16, kind="ExternalInput").ap()
    cosT = nc.dram_tensor("cosT", [128, S], bf16, kind="ExternalInput").ap()
    sinT = nc.dram_tensor("sinT", [128, S], bf16, kind="ExternalInput").ap()
    outT = nc.dram_tensor("outT", [TOK, E], bf16, kind="ExternalOutput").ap()

    with tile.TileContext(nc) as tc, ExitStack() as persist:
        const = persist.enter_context(tc.tile_pool(name="const", bufs=1))
        qt_pool = persist.enter_context(tc.tile_pool(name="qt", bufs=1))
        kt_pool = persist.enter_context(tc.tile_pool(name="kt", bufs=1))
        vaug_pool = persist.enter_context(tc.tile_pool(name="vaug", bufs=1))
        dram = persist.enter_context(
            tc.tile_pool(name="dram", bufs=1, space="DRAM"))

        ident = const.tile([128, 128], bf16, name="ident", tag="ident")
        make_identity(nc, ident[:, :])
        # wq + x chunks go on the sync DGE ring; cos/sin/wo on the scalar
        # ring so the 8 MB wo load does not delay the first matmul.
        wq_sb = const.tile([128, ET, QH], bf16, name="wq_sb", tag="wq")
        nc.sync.dma_start(wq_sb[:, :, :],
                          wqT.rearrange("(t p) o -> p t o", p=128))
        wkv_sb = const.tile([128, ET, KVD], bf16, name="wkv_sb", tag="wkv")
        nc.sync.dma_start(wkv_sb[:, :, :],
                          wkvT.rearrange("(t p) o -> p t o", p=128))
        cos_sb = const.tile([128, S], bf16, name="cos_sb", tag="cos")
        nc.scalar.dma_start(cos_sb[:, :], cosT)
        sin_sb = const.tile([128, S], bf16, name="sin_sb", tag="sin")
        nc.scalar.dma_start(sin_sb[:, :], sinT)
        wo_sb = const.tile([128, ET, E], bf16, name="wo_sb", tag="wo")
        nc.scalar.dma_start(wo_sb[:, :, :],
                            woT.rearrange("(t p) o -> p t o", p=128))
        # triangular causal mask for the diagonal 128-block, dup for 2 heads
        tri = const.tile([128, 2, 128], bf16, name="tri", tag="tri")
        nc.gpsimd.memset(tri[:, :, :], 1.0)
        nc.gpsimd.affine_select(
            out=tri[:, :, :], in_=tri[:, :, :],
            pattern=[[0, 2], [1, 128]], compare_op=mybir.AluOpType.is_ge,
            fill=0.0, base=0, channel_multiplier=-1)

        # persistent activations
        qt_sb = qt_pool.tile([64, HPC, NI], bf16, name="qt", tag="qt")
        kt_sb = kt_pool.tile([64, NI], bf16, name="kt", tag="kt")
        vaug = [vaug_pool.tile([128, 2 * HD], bf16, name=f"va{k}",
                               tag=f"va{k}")
                for k in range(NKTILES)]
        for k in range(NKTILES):
            nc.vector.memset(vaug[k][:, :], 0.0)
            nc.vector.memset(vaug[k][:, 0:1], 1.0)

        # collective buffers: per head-pair m, [slice, 128 rows, 512 tokens]
        cc_in = [dram.tile([NCORES, 128, TOK], bf16, name=f"cc_in{m}",
                           tag=f"ccin{m}") for m in range(2)]
        cc_out = [dram.tile([NCORES, 128, TOK], bf16, name=f"cc_out{m}",
                            tag=f"ccout{m}") for m in range(2)]

        # ---- phase 1: QKV projection + RoPE + V transpose
        with ExitStack() as ph1:
            xt_pool = ph1.enter_context(tc.tile_pool(name="xt", bufs=3))
            proj_ps = ph1.enter_context(
                tc.tile_pool(name="proj_ps", bufs=3, space="PSUM"))
            vt_ps_pool = ph1.enter_context(
                tc.tile_pool(name="vt_ps", bufs=2, space="PSUM"))
            rope_pool = ph1.enter_context(tc.tile_pool(name="rope", bufs=3))
            vs_pool = ph1.enter_context(tc.tile_pool(name="vs", bufs=2))

            def rope(src_ps, parts, s0, dsts):
                # dsts: list of (out_ap, row0) pairs covering src rows
                t1 = rope_pool.tile([128, IC], bf16, name="t1", tag="t1")
                sw = rope_pool.tile([128, IC], bf16, name="sw", tag="sw")
                for h0 in range(0, parts, 64):
                    nc.scalar.copy(sw[h0:h0 + 32, :],
                                   src_ps[h0 + 32:h0 + 64, :])
                    nc.scalar.copy(sw[h0 + 32:h0 + 64, :],
                                   src_ps[h0:h0 + 32, :])
                nc.vector.tensor_mul(t1[:parts, :], src_ps[:parts, :],
                                     cos_sb[:parts, s0:s0 + IC])
                nc.vector.tensor_mul(sw[:parts, :], sw[:parts, :],
                                     sin_sb[:parts, s0:s0 + IC])
                for out_ap, r0 in dsts:
                    nc.vector.tensor_add(out_ap, t1[r0:r0 + 64, :],
                                         sw[r0:r0 + 64, :])

            for ch in range(NI // IC):
                i0 = ch * IC
                s0 = i0 % S
                xt = xt_pool.tile([128, ET, IC], bf16, name="xt", tag="xt")
                nc.sync.dma_start(
                    xt[:, :, :],
                    xT[:, i0:i0 + IC].rearrange("(t p) i -> p t i", p=128))
                for m in range(2):
                    q_ps = proj_ps.tile([128, IC], f32, name="pps",
                                        tag="proj")
                    for t in range(ET):
                        nc.tensor.matmul(
                            q_ps[:, :],
                            wq_sb[:, t, m * 128:(m + 1) * 128],
                            xt[:, t, :],
                            start=(t == 0), stop=(t == ET - 1))
                    rope(q_ps, 128, s0,
                         [(qt_sb[0:64, 2 * m, i0:i0 + IC], 0),
                          (qt_sb[0:64, 2 * m + 1, i0:i0 + IC], 64)])
                kv_ps = proj_ps.tile([128, IC], f32, name="pps", tag="proj")
                for t in range(ET):
                    nc.tensor.matmul(
                        kv_ps[:, :],
                        wkv_sb[:, t, :],
                        xt[:, t, :],
                        start=(t == 0), stop=(t == ET - 1))
                rope(kv_ps, 64, s0, [(kt_sb[0:64, i0:i0 + IC], 0)])
                vs = vs_pool.tile([64, IC], bf16, name="vs", tag="vs")
                nc.scalar.copy(vs[:, :], kv_ps[64:128, :])
                for j in range(IC // 128):
                    kidx = (i0 + j * 128) // 128
                    vt_ps = vt_ps_pool.tile([128, HD], bf16, name="vt",
                                            tag="vt")
                    nc.tensor.transpose(vt_ps[:, :],
                                        vs[:, j * 128:(j + 1) * 128],
                                        ident[0:64, 0:64])
                    nc.vector.tensor_copy(vaug[kidx][:, HD:2 * HD], vt_ps[:, :])

        # ---- phase 3: attention (head-pair outer so the pair-0 AllToAll
        #      overlaps pair-1 compute)
        with ExitStack() as ph3:
            scores_ps = ph3.enter_context(
                tc.tile_pool(name="scores_ps", bufs=2, space="PSUM"))
            ctx_ps_pool = ph3.enter_context(
                tc.tile_pool(name="ctx_ps", bufs=2, space="PSUM"))
            et_pool = ph3.enter_context(tc.tile_pool(name="et", bufs=3))
            rc_pool = ph3.enter_context(tc.tile_pool(name="rc", bufs=2))
            rb_pool = ph3.enter_context(tc.tile_pool(name="rb", bufs=2))
            st_pool = ph3.enter_context(tc.tile_pool(name="st", bufs=2))

            for m in range(2):
                for b in range(B):
                    for qt in range(NQT):
                        sl = b * S + qt * QB
                        nkt = (qt + 1) * (QB // KB)
                        ctx_ps = ctx_ps_pool.tile([128, 2, QB], f32,
                                                  name="ctx", tag="ctx")
                        for kt in range(nkt):
                            j = kt - qt * (QB // KB)
                            kp = b * S + kt * KB
                            s_ps = scores_ps.tile([128, 2, QB], f32,
                                                  name="sps", tag="sps")
                            e_t = et_pool.tile([128, 2, QB], bf16,
                                               name="et", tag="et")
                            if j < 0:
                                for h in range(2):
                                    nc.tensor.matmul(
                                        s_ps[:, h, :],
                                        kt_sb[0:64, kp:kp + KB],
                                        qt_sb[0:64, 2 * m + h, sl:sl + QB],
                                        start=True, stop=True)
                                nc.scalar.activation(e_t[:, :, :],
                                                     s_ps[:, :, :], Exp,
                                                     scale=scale)
                            else:
                                q0 = j * KB
                                for h in range(2):
                                    nc.tensor.matmul(
                                        s_ps[:, h, q0:QB],
                                        kt_sb[0:64, kp:kp + KB],
                                        qt_sb[0:64, 2 * m + h,
                                              sl + q0:sl + QB],
                                        start=True, stop=True)
                                if j > 0:
                                    nc.gpsimd.memset(e_t[:, :, 0:q0], 0.0)
                                nc.scalar.activation(e_t[:, :, q0:QB],
                                                     s_ps[:, :, q0:QB], Exp,
                                                     scale=scale)
                                nc.vector.tensor_mul(
                                    e_t[:, :, q0:q0 + KB],
                                    e_t[:, :, q0:q0 + KB],
                                    tri[:, :, :])
                            for h in range(2):
                                nc.tensor.matmul(
                                    ctx_ps[:, h, :],
                                    vaug[b * SKT + kt][:, :],
                                    e_t[:, h, :],
                                    start=(kt == 0), stop=(kt == nkt - 1))
                        # normalize by the ones-column row + stage for A2A
                        rc = rc_pool.tile([1, 2, QB], f32, name="rc",
                                          tag="rc")
                        nc.vector.reciprocal_approx_fast(
                            rc[:, :, :], ctx_ps[0:1, :, :])
                        rb = rb_pool.tile([64, 2, QB], f32, name="rb",
                                          tag="rb")
                        nc.gpsimd.partition_broadcast(rb[:, :, :],
                                                      rc[:, :, :])
                        stage = st_pool.tile([128, QB], bf16, name="st",
                                             tag="st")
                        nc.vector.tensor_mul(stage[0:64, :],
                                             ctx_ps[HD:2 * HD, 0, :],
                                             rb[:, 0, :])
                        nc.vector.tensor_mul(stage[64:128, :],
                                             ctx_ps[HD:2 * HD, 1, :],
                                             rb[:, 1, :])
                        nc.sync.dma_start(cc_in[m][b * NQT + qt, :, :],
                                          stage[:, :])
                # ---- phase 4: AllToAll for this head-pair
                nc.gpsimd.collective_compute(
                    "AllToAll", mybir.AluOpType.bypass,
                    replica_groups=[list(range(NCORES))],
                    ins=[cc_in[m][:, :, :]],
                    outs=[cc_out[m][:, :, :]])

        # ---- phase 5: output projection for this core's 512-token slice
        with ExitStack() as ph5:
            ct_pool = ph5.enter_context(tc.tile_pool(name="ct", bufs=1))
            out_ps_pool = ph5.enter_context(
                tc.tile_pool(name="out_ps", bufs=2, space="PSUM"))
            ob_pool = ph5.enter_context(tc.tile_pool(name="ob", bufs=2))

            ct = [ct_pool.tile([128, NCORES, TOK], bf16, name=f"ct{m}",
                               tag=f"ct{m}") for m in range(2)]
            for m in range(2):
                for dh in range(2):
                    d0 = dh * (NCORES // 2)
                    nc.sync.dma_start(
                        ct[m][:, d0:d0 + NCORES // 2, :],
                        cc_out[m][d0:d0 + NCORES // 2].rearrange(
                            "s p n -> p s n"))

            out_ps = [out_ps_pool.tile([128, E], f32, name=f"op{c}",
                                       tag="ops") for c in range(4)]
            order = [(0, 0), (1, 0), (0, 1), (1, 1), (2, 0), (2, 1),
                     (3, 0), (3, 1)]
            done = set()
            for c, m in order:
                for d in range(NCORES):
                    for o in range(E // 512):
                        nc.tensor.matmul(
                            out_ps[c][:, o * 512:(o + 1) * 512],
                            ct[m][:, d, c * 128:(c + 1) * 128],
                            wo_sb[:, 2 * d + m, o * 512:(o + 1) * 512],
                            start=(m == 0 and d == 0),
                            stop=(m == 1 and d == NCORES - 1))
                if m == 1:
                    ob = ob_pool.tile([128, E], bf16, name="ob", tag="ob")
                    nc.scalar.copy(ob[:, 0:E // 2], out_ps[c][:, 0:E // 2])
                    nc.vector.tensor_copy(ob[:, E // 2:E],
                                          out_ps[c][:, E // 2:E])
                    nc.sync.dma_start(outT[c * 128:(c + 1) * 128, :],
                                      ob[:, :])
                    done.add(c)
            assert done == {0, 1, 2, 3}

    nc.compile()
    return nc


def make_in_maps(cfg, x, cos, sin, Wq, Wk, Wv, Wo):
    """Host-side prep: transpose/slice full inputs into per-core maps."""
    import ml_dtypes
    B, S, E = cfg["B"], cfg["S"], cfg["E"]
    NH, NKV, HD, NCORES = cfg["NH"], cfg["NKV"], cfg["HD"], cfg["ncores"]
    HPC = NH // NCORES
    QH = HPC * HD
    KVPC = NKV // NCORES
    bf = ml_dtypes.bfloat16

    x = np.asarray(x, dtype=np.float32)
    cos = np.asarray(cos, dtype=np.float32)
    sin = np.asarray(sin, dtype=np.float32)
    Wq = np.asarray(Wq, dtype=np.float32)
    Wk = np.asarray(Wk, dtype=np.float32)
    Wv = np.asarray(Wv, dtype=np.float32)
    Wo = np.asarray(Wo, dtype=np.float32)

    xT = np.ascontiguousarray(x.reshape(B * S, E).T.astype(bf))
    cos_t = cos.T[:HD]                        # [64, S]
    cosT = np.ascontiguousarray(
        np.concatenate([cos_t, cos_t], axis=0).astype(bf))
    sin_t = sin.T[:HD].copy()
    sin_t[:HD // 2] *= -1.0                   # signed sin for rotate-half
    sinT = np.ascontiguousarray(
        np.concatenate([sin_t, sin_t], axis=0).astype(bf))
    woT = np.ascontiguousarray(Wo.T.astype(bf))  # full [E_in, E_out]

    in_maps = []
    for c in range(NCORES):
        qsl = slice(c * QH, (c + 1) * QH)
        ksl = slice(c * KVPC * HD, (c + 1) * KVPC * HD)
        wq = np.ascontiguousarray(Wq[qsl, :].T.astype(bf))
        wkv = np.ascontiguousarray(
            np.concatenate([Wk[ksl, :].T, Wv[ksl, :].T], axis=1).astype(bf))
        in_maps.append(dict(xT=xT, wqT=wq, wkvT=wkv, woT=woT,
                            cosT=cosT, sinT=sinT))
    return in_maps


def assemble_output(cfg, results):
    B, S, E = cfg["B"], cfg["S"], cfg["E"]
    out = np.concatenate([np.asarray(r["outT"]) for r in results], axis=0)
    return np.ascontiguousarray(out.astype(np.float32).reshape(B, S, E))


def kernel(x, mask, cos, sin, Wq, Wk, Wv, Wo):
    global LAST_RESULTS, _CACHED_NC
    _ensure_concourse()
    from concourse import bass_utils

    cfg = FULL_CFG
    if _CACHED_NC is None:
        _CACHED_NC = build_gqa(cfg)
    nc = _CACHED_NC
    in_maps = make_in_maps(cfg, x, cos, sin, Wq, Wk, Wv, Wo)
    res = bass_utils.run_bass_kernel_spmd(
        nc, in_maps, core_ids=list(range(cfg["ncores"])))
    LAST_RESULTS = res
    return assemble_output(cfg, res.results)
